# revision 55
# baseline (speedup 1.0000x reference)
"""Trainium2 Bass kernel for NewExpressionAttentionLayer (sparse gated attention).

Math (per batch b):
  fused = concat(gene, expr) @ W_fused
  Q = split(fused @ (W_Q*scale)); K = split(fused @ W_K); V = split(expr @ W_V)
  t = (Q K^T) * M          (scale folded into W_Q; M = gate)
  p = exp(t)               (softmax without max-subtraction; |t| <~ 6)
  pm = p * M
  A_bar = pm / sum_k(pm)   (softmax Z cancels; EPS term is O(1e-8) relative -> dropped)
  out = (A_bar @ V) @ W_O + b_O

Sharding: 8 cores = 4 batches x 2 query-halves. Each core computes its batch's
projections over all S (needed for K/V) and attention for its 1024 query rows.
For the second query half, the host permutes the sequence axis (swap halves) so
the device program always attends queries s[0:1024] — sums over k are
permutation-invariant.

Device layout is feature-major ("transposed"): activations [feat, seq] so the
PE (which contracts along partitions) needs no on-device transposes. The host
supplies X^T and M^T slices. Scores are computed transposed: scoresT[k, q] =
K^T_h.T @ Q^T_h.

Engine constraints that shape the schedule: only DVE and Act can read PSUM
(GPSIMD/Pool is SBUF-only, DMA cannot touch PSUM, matmul output is fp32
PSUM). So u = scores*M runs on DVE (f32, 1024-wide kt-pair ops); exp on
Act; pm = e*M (all-bf16 SBUF, 2x mode on DVE) alternates per head parity
between DVE and Pool; the per-query normalization of the output projection
alternates between a DVE scalar_tensor_tensor and an Act
per-partition-scale + Pool add. Per-head Z sums ride as a ones-column in
the V matmul, land on one PSUM row, and transpose to q-partition columns
via contraction-1 PE matmuls.

Two heads' chains are interleaved, AV matmuls are emitted several score
steps late (uniform lag, FIFO) so slow Pool multiplies never block the PE
wait queue, and a retired head pair's epilogue (copies, Z transpose,
output projection + normalize) is split into pieces drained one per step.
Head (0, qc0)'s chain is additionally interleaved into the projection
phase, where DVE/Act/Pool are otherwise idle.
"""

import sys

sys.path.insert(0, "/opt/trn_rl_repo")

import numpy as np

B, S, D = 4, 2048, 512
H, HD = 8, 64
SQ = S // 2          # query rows per core
KT_TILES = S // 128  # 16 k partition tiles
N_KTP = KT_TILES // 2  # 8 kt pairs
QC_W = 512           # q chunk width
N_QC = SQ // QC_W    # 2
SC_W = 256           # s chunk width for projections
N_SC = S // SC_W     # 8
NQT = QC_W // 128    # 4 query tiles per chunk

AV_LAG = 11           # score steps between pm emission and its AV matmuls

_PROG = None


def _build_program():
    from concourse import bacc, mybir
    import concourse.tile as tile

    f32 = mybir.dt.float32
    f32r = mybir.dt.float32r
    bf16 = mybir.dt.bfloat16
    Exp = mybir.ActivationFunctionType.Exp
    Copy = mybir.ActivationFunctionType.Copy
    MUL = mybir.AluOpType.mult
    ADD = mybir.AluOpType.add

    nc = bacc.Bacc("TRN2", target_bir_lowering=False, debug=False, num_devices=8)

    XT = nc.dram_tensor("XT", [2 * D, S], f32r, kind="ExternalInput").ap()
    MT = nc.dram_tensor("MT", [S, SQ], bf16, kind="ExternalInput").ap()
    WF = nc.dram_tensor("WF", [2 * D, D], f32r, kind="ExternalInput").ap()
    WQ = nc.dram_tensor("WQ", [D, D], f32r, kind="ExternalInput").ap()
    WK = nc.dram_tensor("WK", [D, D], f32r, kind="ExternalInput").ap()
    WV = nc.dram_tensor("WV", [D, D], f32r, kind="ExternalInput").ap()
    WO = nc.dram_tensor("WO", [D, D], bf16, kind="ExternalInput").ap()
    OUT = nc.dram_tensor("OUT", [SQ, D], f32, kind="ExternalOutput").ap()

    with tile.TileContext(nc) as tc:
        with (
            tc.tile_pool(name="misc", bufs=1) as misc,
            tc.tile_pool(name="kqv", bufs=1) as kqv,
            tc.tile_pool(name="mtp", bufs=1) as mtp,
            tc.tile_pool(name="att1", bufs=1) as att1,
            tc.tile_pool(name="att2", bufs=1) as att2,
            tc.tile_pool(name="wku", bufs=2) as wku,
            tc.tile_pool(name="wke", bufs=2) as wke,
            tc.tile_pool(name="wkp", bufs=2) as wkp,
            tc.tile_pool(name="psB0", bufs=1, space="PSUM") as psB0,
            tc.tile_pool(name="psAV0", bufs=1, space="PSUM") as psAV0,
        ):
            onecol = misc.tile([128, 1], bf16)
            nc.vector.memset(onecol, 1.0)
            one32 = misc.tile([1, 1], f32)
            nc.vector.memset(one32, 1.0)
            wo_sb = misc.tile([128, 4, D], bf16)

            kt_sb = kqv.tile([128, 4, S], f32r)     # K^T  [d, s]
            qt_sb = kqv.tile([128, 4, SQ], f32r)    # Q^T  [d, q]
            v_sb = kqv.tile([128, KT_TILES, H, HD + 1], bf16)  # V + ones col
            nc.vector.tensor_copy(
                v_sb[:, :, :, HD : HD + 1],
                onecol[:, None, :].broadcast_to([128, KT_TILES, H, 1]),
            )

            mt_r = MT.rearrange("(t p) q -> p t q", p=128)
            # the gate streams through a 4-deep ring of quarter tiles
            # (4 kt tiles each) so qc1's quarters prefetch while qc0 retires
            mt_quarters = {}
            qsls = [slice(0, QC_W), slice(QC_W, 2 * QC_W)]

            def load_mt_quarter(qc, q4):
                t = mtp.tile(
                    [128, 4, QC_W], bf16, tag="mt", name=f"mt{qc}_{q4}", bufs=4
                )
                nc.sync.dma_start(
                    out=t, in_=mt_r[:, q4 * 4 : (q4 + 1) * 4, qsls[qc]]
                )
                mt_quarters[(qc, q4)] = t

            def make_ctx(qc):
                outt = att1.tile(
                    [128, 4, QC_W], bf16, tag="outt", name=f"outt{qc}"
                )
                zr = att1.tile([1, H, QC_W], f32, tag="zr", name=f"zr{qc}")
                ps_t = [None]  # allocated lazily (PSUM pool opens post-proj)
                invt = att2.tile(
                    [128, NQT, H], f32, tag="invt", name=f"invt{qc}"
                )
                fin = [
                    att2.tile([128, D], f32, tag=f"fin{qtl}", name=f"fin{qc}_{qtl}")
                    for qtl in range(NQT)
                ]
                return (outt, zr, ps_t, invt, fin)

            # score-ring pools per head parity; head-1 ring + outproj PSUM
            # open only after the projection pools close (PSUM budget: during
            # proj psP(5) + psB0(2) + psAV0(1) = 8; after proj psB0(2) +
            # psAV0(1) + psB1(2) + psAV1(1) + psO(2) = 8)
            pools = {"sc0": psB0, "av0": psAV0, "sc1": None, "av1": None,
                     "o": None, "tp": None}

            # work FIFOs; the global step counter advances per score step
            sched = {"iter": 0, "av": [], "epi": [], "stt": 0}

            def drain():
                it = sched["iter"]
                while sched["av"] and sched["av"][0][0] <= it:
                    sched["av"].pop(0)[1]()
                if sched["epi"] and sched["epi"][0][0] <= it:
                    sched["epi"].pop(0)[1]()

            def flush_all():
                for _, f in sched["av"]:
                    f()
                sched["av"] = []
                for _, f in sched["epi"]:
                    f()
                sched["epi"] = []

            def mk_av(av_ps, ph, kt, ppm, i):
                def f():
                    nc.tensor.matmul(
                        av_ps, v_sb[:, kt, ph, :], ppm[:, i, :],
                        start=(kt == 0), stop=(kt == KT_TILES - 1),
                    )
                return f

            def att_step(qc, hp, j, ktp, av_ps):
                qsl = qsls[qc]
                p0, p1 = (0, 64) if j == 0 else (64, 128)
                h = 2 * hp + j
                off = (2 * ktp) % 4
                mt2 = mt_quarters[(qc, ktp // 2)][:, off : off + 2, :]
                scj = pools[f"sc{j}"].tile(
                    [128, 2, QC_W], f32, tag=f"sc{j}", name=f"sc{j}"
                )
                for i in range(2):
                    kt = 2 * ktp + i
                    nc.tensor.matmul(
                        scj[:, i, :],
                        kt_sb[p0:p1, hp, kt * 128 : (kt + 1) * 128],
                        qt_sb[p0:p1, hp, qsl],
                        start=True, stop=True,
                    )
                sched["iter"] += 1
                drain()
                u = wku.tile([128, 2, QC_W], f32, tag=f"u{j}", name=f"u{j}")
                nc.vector.tensor_mul(u, scj, mt2)
                e = wke.tile([128, 2, QC_W], bf16, tag=f"e{j}", name=f"e{j}")
                nc.scalar.activation(e, u, Exp)
                pm = wkp.tile([128, 2, QC_W], bf16, tag=f"pm{j}", name=f"pm{j}")
                # pm per head parity: even heads on DVE (2x bf16), odd on Pool
                if j == 0:
                    nc.vector.tensor_mul(pm, e, mt2)
                else:
                    nc.gpsimd.tensor_mul(pm, e, mt2)
                due = sched["iter"] + AV_LAG
                for i in range(2):
                    sched["av"].append((due, mk_av(av_ps, h, 2 * ktp + i, pm, i)))

            def mk_epilogue(qc, hp, ps_av0, ps_av1, ctx):
                outt, zr, ps_t, invt, fin = ctx

                def piece0():
                    if ps_t[0] is None:
                        ps_t[0] = pools["tp"].tile(
                            [128, NQT, H], f32, tag="tp", name=f"ps_t{qc}"
                        )
                    pst = ps_t[0]
                    for j, av_ps in ((0, ps_av0), (1, ps_av1)):
                        h = 2 * hp + j
                        nc.scalar.activation(
                            outt[j * 64 : j * 64 + 64, hp, :], av_ps[0:HD, :], Copy
                        )
                        nc.scalar.activation(
                            zr[0:1, h, :], av_ps[HD : HD + 1, :], Copy
                        )
                    # row->column transpose via contraction-1 matmuls
                    for j in range(2):
                        h = 2 * hp + j
                        for qtl in range(NQT):
                            nc.tensor.matmul(
                                pst[:, qtl, h : h + 1],
                                zr[0:1, h, qtl * 128 : (qtl + 1) * 128],
                                one32, start=True, stop=True,
                            )
                    nc.vector.reciprocal(
                        invt[:, :, 2 * hp : 2 * hp + 2],
                        pst[:, :, 2 * hp : 2 * hp + 2],
                    )

                pieces = [piece0]
                for j in range(2):
                    for qtl in range(NQT):
                        def piece(j=j, qtl=qtl):
                            h = 2 * hp + j
                            hoff = j * 64
                            qt_g = qc * NQT + qtl
                            ps_o = pools["o"].tile(
                                [128, D], f32, tag="o", name="ps_o"
                            )
                            nc.tensor.matmul(
                                ps_o,
                                outt[hoff : hoff + 64, hp,
                                     qtl * 128 : (qtl + 1) * 128],
                                wo_sb[hoff : hoff + 64, hp, :],
                                start=True, stop=True,
                            )
                            inv = invt[:, qtl, h : h + 1]
                            via_act = sched["stt"] % 2 == 1
                            sched["stt"] += 1
                            if h == 0:
                                if via_act:
                                    nc.scalar.mul(fin[qtl], ps_o, inv)
                                else:
                                    nc.vector.tensor_scalar_mul(
                                        fin[qtl], ps_o, inv
                                    )
                            elif via_act:
                                tmp = att2.tile(
                                    [128, D], bf16, tag="tmp", name="tmp"
                                )
                                nc.scalar.mul(tmp, ps_o, inv)
                                nc.gpsimd.tensor_add(fin[qtl], fin[qtl], tmp)
                            else:
                                nc.vector.scalar_tensor_tensor(
                                    out=fin[qtl], in0=ps_o, scalar=inv,
                                    in1=fin[qtl], op0=MUL, op1=ADD,
                                )
                            if h == H - 1:
                                nc.sync.dma_start(
                                    out=OUT[qt_g * 128 : (qt_g + 1) * 128, :],
                                    in_=fin[qtl],
                                )
                        pieces.append(piece)
                return pieces

            # ---------------- projection phase ----------------
            # head (hp0, j0) attention for qc0 interleaves with projection:
            # after chunk c, kt tiles 2c,2c+1 (and matching V rows) exist, so
            # its ktp step c-1 is safe (qt cols for qc0 exist after chunk 1)
            ctx0 = make_ctx(0)
            av00 = psAV0.tile([HD + 1, QC_W], f32, tag="av0", name="av0_00")

            def proj_att_step(c):
                ktp = c - 1
                if ktp < 0 or ktp >= N_KTP - 1:
                    return
                att_step(0, 0, 0, ktp, av00)

            with (
                tc.tile_pool(name="projw", bufs=1) as projw,
                tc.tile_pool(name="xtp", bufs=2) as xtp,
                tc.tile_pool(name="fcp", bufs=1) as fcp,
                tc.tile_pool(name="psP", bufs=2, space="PSUM") as psP,
            ):
                xt_r = XT.rearrange("(t p) s -> p t s", p=128)
                wf_sb = projw.tile([128, 8, D], f32r)
                wf_r = WF.rearrange("(t p) n -> p t n", p=128)
                # first fused matmul needs wf[t] + xt0[t] in stream order
                xt_c0 = xtp.tile([128, 8, SC_W], f32r, tag="xt")
                for t in range(8):
                    nc.sync.dma_start(out=wf_sb[:, t, :], in_=wf_r[:, t, :])
                    nc.sync.dma_start(out=xt_c0[:, t, :], in_=xt_r[:, t, 0:SC_W])
                wk_sb = projw.tile([128, 4, D], f32r)
                nc.sync.dma_start(out=wk_sb, in_=WK.rearrange("(t p) n -> p t n", p=128))
                wq_sb = projw.tile([128, 4, D], f32r)
                nc.sync.dma_start(out=wq_sb, in_=WQ.rearrange("(t p) n -> p t n", p=128))
                wv_sb = projw.tile([128, 4, D], f32r)
                nc.sync.dma_start(out=wv_sb, in_=WV.rearrange("(t p) n -> p t n", p=128))
                xt_c1 = xtp.tile([128, 8, SC_W], f32r, tag="xt")
                nc.sync.dma_start(out=xt_c1, in_=xt_r[:, :, SC_W : 2 * SC_W])
                nc.sync.dma_start(out=wo_sb, in_=WO.rearrange("(t p) n -> p t n", p=128))

                for sc in range(N_SC):
                    ssl = slice(sc * SC_W, (sc + 1) * SC_W)
                    if sc == 0:
                        xt_c = xt_c0
                    elif sc == 1:
                        xt_c = xt_c1
                    else:
                        xt_c = xtp.tile([128, 8, SC_W], f32r, tag="xt")
                        nc.sync.dma_start(out=xt_c, in_=xt_r[:, :, ssl])
                    # stream a quarter of the qc0 gate right behind each
                    # even chunk's xt load (quarter q4 feeds ktp 2*q4+..)
                    if sc % 2 == 0 and sc < 8:
                        load_mt_quarter(0, sc // 2)

                    fc = fcp.tile([128, 4, SC_W], f32r, tag="fc")
                    for dt in range(4):
                        ps = psP.tile([128, SC_W], f32, tag="mm", bufs=3)
                        for t in range(8):
                            nc.tensor.matmul(
                                ps, wf_sb[:, t, dt * 128 : (dt + 1) * 128],
                                xt_c[:, t, :], start=(t == 0), stop=(t == 7),
                            )
                        nc.scalar.activation(fc[:, dt, :], ps, Copy)

                    # K^T (all s) and Q^T (first half = query rows)
                    for w_sb, dst in (
                        (wk_sb, kt_sb[:, :, ssl]),
                        (wq_sb, qt_sb[:, :, ssl] if sc * SC_W < SQ else None),
                    ):
                        if dst is None:
                            continue
                        for ot in range(4):
                            ps = psP.tile([128, SC_W], f32, tag="mm", bufs=3)
                            for dt in range(4):
                                nc.tensor.matmul(
                                    ps, w_sb[:, dt, ot * 128 : (ot + 1) * 128],
                                    fc[:, dt, :], start=(dt == 0), stop=(dt == 3),
                                )
                            nc.scalar.activation(dst[:, ot, :], ps, Copy)

                    # V rows for this s chunk (expr = contraction tiles 4..7)
                    for st in range(SC_W // 128):
                        sidx = sc * (SC_W // 128) + st
                        ps = psP.tile([128, D], f32, tag="mmv")
                        for dt in range(4):
                            nc.tensor.matmul(
                                ps, xt_c[:, 4 + dt, st * 128 : (st + 1) * 128],
                                wv_sb[:, dt, :], start=(dt == 0), stop=(dt == 3),
                            )
                        nc.scalar.activation(
                            v_sb[:, sidx, :, 0:HD],
                            ps.rearrange("p (h d) -> p h d", h=H),
                            Copy,
                        )

                    proj_att_step(sc)

            # ---------------- attention phase (remainder) ----------------
            with (
                tc.tile_pool(name="psB1", bufs=1, space="PSUM") as psB1,
                tc.tile_pool(name="psAV1", bufs=1, space="PSUM") as psAV1,
                tc.tile_pool(name="psO", bufs=1, space="PSUM") as psO,
                tc.tile_pool(name="psC", bufs=1, space="PSUM") as psC_,
            ):
                pools["tp"] = psC_
                pools["sc1"] = psB1
                pools["av1"] = psAV1
                pools["o"] = psO

                # finish head (0,0): last ktp step
                att_step(0, 0, 0, N_KTP - 1, av00)

                ctxs = {0: ctx0}
                for qc in range(N_QC):
                    if qc not in ctxs:
                        ctxs[qc] = make_ctx(qc)
                    ctx = ctxs[qc]
                    for hp in range(4):
                        done0 = qc == 0 and hp == 0
                        if done0:
                            av_ps0 = av00
                        else:
                            av_ps0 = psAV0.tile(
                                [HD + 1, QC_W], f32, tag="av0",
                                name=f"av0_{qc}{hp}",
                            )
                        av_ps1 = psAV1.tile(
                            [HD + 1, QC_W], f32, tag="av1", name=f"av1_{qc}{hp}"
                        )
                        for ktp in range(N_KTP):
                            for j in range(2):
                                if j == 0 and done0:
                                    continue
                                att_step(
                                    qc, hp, j, ktp,
                                    av_ps0 if j == 0 else av_ps1,
                                )
                            if qc == 0 and hp == 3 and ktp % 2 == 1:
                                load_mt_quarter(1, ktp // 2)
                        # epilogue must drain after this head pair's last
                        # (lagged) AV matmuls have been emitted
                        epi_due = sched["iter"] + AV_LAG + 1
                        sched["epi"].extend(
                            (epi_due, p)
                            for p in mk_epilogue(qc, hp, av_ps0, av_ps1, ctx)
                        )
                flush_all()

    nc.compile()
    return nc


def _get_prog():
    global _PROG
    if _PROG is None:
        _PROG = _build_program()
    return _PROG


def kernel(**inputs) -> np.ndarray:
    from concourse.bass_utils import run_bass_kernel_spmd
    from concourse import mybir

    bf16np = mybir.dt.np(mybir.dt.bfloat16)

    f = lambda k: np.asarray(inputs[k], dtype=np.float32)
    gene, expr, M = f("gene_emb"), f("expr_emb"), f("M")
    W_fused = f("W_fused")
    W_Q, W_K, W_V, W_O = f("W_Q"), f("W_K"), f("W_V"), f("W_O")
    b_O = f("b_O")

    scale = np.float32(HD ** -0.5)
    weights = dict(
        WF=np.ascontiguousarray(W_fused),
        WQ=np.ascontiguousarray(W_Q * scale),
        WK=np.ascontiguousarray(W_K),
        WV=np.ascontiguousarray(W_V),
        WO=np.ascontiguousarray(W_O).astype(bf16np),
    )

    nc = _get_prog()

    in_maps = []
    for c in range(8):
        b, qh = c // 2, c % 2
        xt = np.concatenate([gene[b], expr[b]], axis=1).T  # [1024, 2048]
        mt = M[b, qh * SQ : (qh + 1) * SQ, :].T            # [2048, 1024]
        if qh == 1:
            # permute sequence so this core's queries are s[0:1024]
            xt = np.concatenate([xt[:, SQ:], xt[:, :SQ]], axis=1)
            mt = np.concatenate([mt[SQ:], mt[:SQ]], axis=0)
        in_maps.append(
            dict(
                XT=np.ascontiguousarray(xt),
                MT=np.ascontiguousarray(mt).astype(bf16np),
                **weights,
            )
        )

    res = run_bass_kernel_spmd(nc, in_maps, core_ids=list(range(8)))

    out = np.empty((B, S, D), dtype=np.float32)
    for c in range(8):
        b, qh = c // 2, c % 2
        out[b, qh * SQ : (qh + 1) * SQ, :] = res.results[c]["OUT"] + b_O[None, :]
    return out


# revision 63
# speedup vs baseline: 1.0051x; 1.0051x over previous
"""Trainium2 Bass kernel for NewExpressionAttentionLayer (sparse gated attention).

Math (per batch b):
  fused = concat(gene, expr) @ W_fused
  Q = split(fused @ (W_Q*scale)); K = split(fused @ W_K); V = split(expr @ W_V)
  t = (Q K^T) * M          (scale folded into W_Q; M = gate)
  p = exp(t)               (softmax without max-subtraction; |t| <~ 6)
  pm = p * M
  A_bar = pm / sum_k(pm)   (softmax Z cancels; EPS term is O(1e-8) relative -> dropped)
  out = (A_bar @ V) @ W_O + b_O

Sharding: 8 cores = 4 batches x 2 query-halves. Each core computes its batch's
projections over all S (needed for K/V) and attention for its 1024 query rows.
For the second query half, the host permutes the sequence axis (swap halves) so
the device program always attends queries s[0:1024] — sums over k are
permutation-invariant.

Device layout is feature-major ("transposed"): activations [feat, seq] so the
PE (which contracts along partitions) needs no on-device transposes. The host
supplies X^T and M^T slices. Scores are computed transposed: scoresT[k, q] =
K^T_h.T @ Q^T_h.

Engine constraints that shape the schedule: only DVE and Act can read PSUM
(GPSIMD/Pool is SBUF-only, DMA cannot touch PSUM, matmul output is fp32
PSUM). So u = scores*M runs on DVE (f32, 1024-wide kt-pair ops); exp on
Act; pm = e*M (all-bf16 SBUF, 2x mode on DVE) alternates per head parity
between DVE and Pool; the per-query normalization of the output projection
alternates between a DVE scalar_tensor_tensor and an Act
per-partition-scale + Pool add. Per-head Z sums ride as a ones-column in
the V matmul, land on one PSUM row, and transpose to q-partition columns
via contraction-1 PE matmuls.

Two heads' chains are interleaved, AV matmuls are emitted several score
steps late (uniform lag, FIFO) so slow Pool multiplies never block the PE
wait queue, and a retired head pair's epilogue (copies, Z transpose,
output projection + normalize) is split into pieces drained one per step.
Head (0, qc0)'s chain is additionally interleaved into the projection
phase, where DVE/Act/Pool are otherwise idle.
"""

import sys

sys.path.insert(0, "/opt/trn_rl_repo")

import numpy as np

B, S, D = 4, 2048, 512
H, HD = 8, 64
SQ = S // 2          # query rows per core
KT_TILES = S // 128  # 16 k partition tiles
N_KTP = KT_TILES // 2  # 8 kt pairs
QC_W = 512           # q chunk width
N_QC = SQ // QC_W    # 2
SC_W = 256           # s chunk width for projections
N_SC = S // SC_W     # 8
NQT = QC_W // 128    # 4 query tiles per chunk

AV_LAG = 11           # score steps between pm emission and its AV matmuls

_PROG = None


def _build_program():
    from concourse import bacc, mybir
    import concourse.tile as tile

    f32 = mybir.dt.float32
    f32r = mybir.dt.float32r
    bf16 = mybir.dt.bfloat16
    Exp = mybir.ActivationFunctionType.Exp
    Copy = mybir.ActivationFunctionType.Copy
    MUL = mybir.AluOpType.mult
    ADD = mybir.AluOpType.add

    nc = bacc.Bacc("TRN2", target_bir_lowering=False, debug=False, num_devices=8)

    XT = nc.dram_tensor("XT", [2 * D, S], f32r, kind="ExternalInput").ap()
    MT = nc.dram_tensor("MT", [S, SQ], bf16, kind="ExternalInput").ap()
    WF = nc.dram_tensor("WF", [2 * D, D], f32r, kind="ExternalInput").ap()
    WQ = nc.dram_tensor("WQ", [D, D], f32r, kind="ExternalInput").ap()
    WK = nc.dram_tensor("WK", [D, D], f32r, kind="ExternalInput").ap()
    WV = nc.dram_tensor("WV", [D, D], f32r, kind="ExternalInput").ap()
    WO = nc.dram_tensor("WO", [D, D], bf16, kind="ExternalInput").ap()
    OUT = nc.dram_tensor("OUT", [SQ, D], f32, kind="ExternalOutput").ap()

    with tile.TileContext(nc) as tc:
        with (
            tc.tile_pool(name="misc", bufs=1) as misc,
            tc.tile_pool(name="kqv", bufs=1) as kqv,
            tc.tile_pool(name="mtp", bufs=1) as mtp,
            tc.tile_pool(name="att1", bufs=1) as att1,
            tc.tile_pool(name="att2", bufs=1) as att2,
            tc.tile_pool(name="wku", bufs=3) as wku,
            tc.tile_pool(name="wke", bufs=2) as wke,
            tc.tile_pool(name="wkp", bufs=2) as wkp,
            tc.tile_pool(name="psB0", bufs=1, space="PSUM") as psB0,
            tc.tile_pool(name="psAV0", bufs=1, space="PSUM") as psAV0,
        ):
            onecol = misc.tile([128, 1], bf16)
            nc.vector.memset(onecol, 1.0)
            one32 = misc.tile([1, 1], f32)
            nc.vector.memset(one32, 1.0)
            wo_sb = misc.tile([128, 4, D], bf16)

            kt_sb = kqv.tile([128, 4, S], bf16)     # K^T  [d, s]
            qt_sb = kqv.tile([128, 4, SQ], bf16)    # Q^T  [d, q]
            v_sb = kqv.tile([128, KT_TILES, H, HD + 1], bf16)  # V + ones col
            nc.vector.tensor_copy(
                v_sb[:, :, :, HD : HD + 1],
                onecol[:, None, :].broadcast_to([128, KT_TILES, H, 1]),
            )

            mt_r = MT.rearrange("(t p) q -> p t q", p=128)
            # the gate streams through a 4-deep ring of quarter tiles
            # (4 kt tiles each) so qc1's quarters prefetch while qc0 retires
            mt_quarters = {}
            qsls = [slice(0, QC_W), slice(QC_W, 2 * QC_W)]

            def load_mt_quarter(qc, q4):
                t = mtp.tile(
                    [128, 4, QC_W], bf16, tag="mt", name=f"mt{qc}_{q4}", bufs=4
                )
                nc.sync.dma_start(
                    out=t, in_=mt_r[:, q4 * 4 : (q4 + 1) * 4, qsls[qc]]
                )
                mt_quarters[(qc, q4)] = t

            def make_ctx(qc):
                outt = att1.tile(
                    [128, 4, QC_W], bf16, tag="outt", name=f"outt{qc}"
                )
                zr = att1.tile([1, H, QC_W], f32, tag="zr", name=f"zr{qc}")
                ps_t = [None]  # allocated lazily (PSUM pool opens post-proj)
                invt = att2.tile(
                    [128, NQT, H], f32, tag="invt", name=f"invt{qc}"
                )
                fin = [
                    att2.tile([128, D], f32, tag=f"fin{qtl}", name=f"fin{qc}_{qtl}")
                    for qtl in range(NQT)
                ]
                return (outt, zr, ps_t, invt, fin)

            # score-ring pools per head parity; head-1 ring + outproj PSUM
            # open only after the projection pools close (PSUM budget: during
            # proj psP(5) + psB0(2) + psAV0(1) = 8; after proj psB0(2) +
            # psAV0(1) + psB1(2) + psAV1(1) + psO(2) = 8)
            pools = {"sc0": psB0, "av0": psAV0, "sc1": None, "av1": None,
                     "o": None, "tp": None}

            # work FIFOs; the global step counter advances per score step
            sched = {"iter": 0, "av": [], "epi": [], "stt": 0}

            def drain():
                it = sched["iter"]
                while sched["av"] and sched["av"][0][0] <= it:
                    sched["av"].pop(0)[1]()
                if sched["epi"] and sched["epi"][0][0] <= it:
                    sched["epi"].pop(0)[1]()

            def flush_all():
                for _, f in sched["av"]:
                    f()
                sched["av"] = []
                for _, f in sched["epi"]:
                    f()
                sched["epi"] = []

            def mk_av(av_ps, ph, kt, ppm, i):
                def f():
                    nc.tensor.matmul(
                        av_ps, v_sb[:, kt, ph, :], ppm[:, i, :],
                        start=(kt == 0), stop=(kt == KT_TILES - 1),
                    )
                return f

            def att_step(qc, hp, j, ktp, av_ps):
                qsl = qsls[qc]
                p0, p1 = (0, 64) if j == 0 else (64, 128)
                h = 2 * hp + j
                off = (2 * ktp) % 4
                mt2 = mt_quarters[(qc, ktp // 2)][:, off : off + 2, :]
                scj = pools[f"sc{j}"].tile(
                    [128, 2, QC_W], f32, tag=f"sc{j}", name=f"sc{j}"
                )
                for i in range(2):
                    kt = 2 * ktp + i
                    nc.tensor.matmul(
                        scj[:, i, :],
                        kt_sb[p0:p1, hp, kt * 128 : (kt + 1) * 128],
                        qt_sb[p0:p1, hp, qsl],
                        start=True, stop=True,
                    )
                sched["iter"] += 1
                drain()
                u = wku.tile([128, 2, QC_W], f32, tag=f"u{j}", name=f"u{j}")
                nc.vector.tensor_mul(u, scj, mt2)
                e = wke.tile([128, 2, QC_W], bf16, tag=f"e{j}", name=f"e{j}")
                nc.scalar.activation(e, u, Exp)
                pm = wkp.tile([128, 2, QC_W], bf16, tag=f"pm{j}", name=f"pm{j}")
                # pm per head parity: even heads on DVE (2x bf16), odd on Pool
                if j == 0:
                    nc.vector.tensor_mul(pm, e, mt2)
                else:
                    nc.gpsimd.tensor_mul(pm, e, mt2)
                due = sched["iter"] + AV_LAG
                for i in range(2):
                    sched["av"].append((due, mk_av(av_ps, h, 2 * ktp + i, pm, i)))

            def mk_epilogue(qc, hp, ps_av0, ps_av1, ctx):
                outt, zr, ps_t, invt, fin = ctx

                def piece0():
                    if ps_t[0] is None:
                        ps_t[0] = pools["tp"].tile(
                            [128, NQT, H], f32, tag="tp", name=f"ps_t{qc}"
                        )
                    pst = ps_t[0]
                    for j, av_ps in ((0, ps_av0), (1, ps_av1)):
                        h = 2 * hp + j
                        nc.scalar.activation(
                            outt[j * 64 : j * 64 + 64, hp, :], av_ps[0:HD, :], Copy
                        )
                        nc.scalar.activation(
                            zr[0:1, h, :], av_ps[HD : HD + 1, :], Copy
                        )
                    # row->column transpose via contraction-1 matmuls
                    for j in range(2):
                        h = 2 * hp + j
                        for qtl in range(NQT):
                            nc.tensor.matmul(
                                pst[:, qtl, h : h + 1],
                                zr[0:1, h, qtl * 128 : (qtl + 1) * 128],
                                one32, start=True, stop=True,
                            )
                    nc.vector.reciprocal(
                        invt[:, :, 2 * hp : 2 * hp + 2],
                        pst[:, :, 2 * hp : 2 * hp + 2],
                    )

                pieces = [piece0]
                for j in range(2):
                    for qtl in range(NQT):
                        def piece(j=j, qtl=qtl):
                            h = 2 * hp + j
                            hoff = j * 64
                            qt_g = qc * NQT + qtl
                            ps_o = pools["o"].tile(
                                [128, D], f32, tag="o", name="ps_o"
                            )
                            nc.tensor.matmul(
                                ps_o,
                                outt[hoff : hoff + 64, hp,
                                     qtl * 128 : (qtl + 1) * 128],
                                wo_sb[hoff : hoff + 64, hp, :],
                                start=True, stop=True,
                            )
                            inv = invt[:, qtl, h : h + 1]
                            via_act = sched["stt"] % 2 == 1
                            sched["stt"] += 1
                            if h == 0:
                                if via_act:
                                    nc.scalar.mul(fin[qtl], ps_o, inv)
                                else:
                                    nc.vector.tensor_scalar_mul(
                                        fin[qtl], ps_o, inv
                                    )
                            elif via_act:
                                tmp = att2.tile(
                                    [128, D], bf16, tag="tmp", name="tmp"
                                )
                                nc.scalar.mul(tmp, ps_o, inv)
                                nc.gpsimd.tensor_add(fin[qtl], fin[qtl], tmp)
                            else:
                                nc.vector.scalar_tensor_tensor(
                                    out=fin[qtl], in0=ps_o, scalar=inv,
                                    in1=fin[qtl], op0=MUL, op1=ADD,
                                )
                            if h == H - 1:
                                nc.sync.dma_start(
                                    out=OUT[qt_g * 128 : (qt_g + 1) * 128, :],
                                    in_=fin[qtl],
                                )
                        pieces.append(piece)
                return pieces

            # ---------------- projection phase ----------------
            # head (hp0, j0) attention for qc0 interleaves with projection:
            # after chunk c, kt tiles 2c,2c+1 (and matching V rows) exist, so
            # its ktp step c-1 is safe (qt cols for qc0 exist after chunk 1)
            ctx0 = make_ctx(0)
            av00 = psAV0.tile([HD + 1, QC_W], f32, tag="av0", name="av0_00")

            def proj_att_step(c):
                ktp = c - 1
                if ktp < 0 or ktp >= N_KTP - 1:
                    return
                att_step(0, 0, 0, ktp, av00)

            with (
                tc.tile_pool(name="projw", bufs=1) as projw,
                tc.tile_pool(name="xtp", bufs=2) as xtp,
                tc.tile_pool(name="fcp", bufs=2) as fcp,
                tc.tile_pool(name="psP", bufs=2, space="PSUM") as psP,
            ):
                xt_r = XT.rearrange("(t p) s -> p t s", p=128)
                wf_sb = projw.tile([128, 8, D], f32r)
                wf_r = WF.rearrange("(t p) n -> p t n", p=128)
                # first fused matmul needs wf[t] + xt0[t] in stream order
                xt_c0 = xtp.tile([128, 8, SC_W], f32r, tag="xt")
                for t in range(8):
                    nc.sync.dma_start(out=wf_sb[:, t, :], in_=wf_r[:, t, :])
                    nc.sync.dma_start(out=xt_c0[:, t, :], in_=xt_r[:, t, 0:SC_W])
                wk_sb = projw.tile([128, 4, D], f32r)
                nc.sync.dma_start(out=wk_sb, in_=WK.rearrange("(t p) n -> p t n", p=128))
                wq_sb = projw.tile([128, 4, D], f32r)
                nc.sync.dma_start(out=wq_sb, in_=WQ.rearrange("(t p) n -> p t n", p=128))
                wv_sb = projw.tile([128, 4, D], f32r)
                nc.sync.dma_start(out=wv_sb, in_=WV.rearrange("(t p) n -> p t n", p=128))
                xt_c1 = xtp.tile([128, 8, SC_W], f32r, tag="xt")
                nc.sync.dma_start(out=xt_c1, in_=xt_r[:, :, SC_W : 2 * SC_W])
                nc.sync.dma_start(out=wo_sb, in_=WO.rearrange("(t p) n -> p t n", p=128))

                for sc in range(N_SC):
                    ssl = slice(sc * SC_W, (sc + 1) * SC_W)
                    if sc == 0:
                        xt_c = xt_c0
                    elif sc == 1:
                        xt_c = xt_c1
                    else:
                        xt_c = xtp.tile([128, 8, SC_W], f32r, tag="xt")
                        nc.sync.dma_start(out=xt_c, in_=xt_r[:, :, ssl])
                    # stream a quarter of the qc0 gate right behind each
                    # even chunk's xt load (quarter q4 feeds ktp 2*q4+..)
                    if sc % 2 == 0 and sc < 8:
                        load_mt_quarter(0, sc // 2)

                    fc = fcp.tile([128, 4, SC_W], f32r, tag="fc")
                    for dt in range(4):
                        ps = psP.tile([128, SC_W], f32, tag="mm", bufs=3)
                        for t in range(8):
                            nc.tensor.matmul(
                                ps, wf_sb[:, t, dt * 128 : (dt + 1) * 128],
                                xt_c[:, t, :], start=(t == 0), stop=(t == 7),
                            )
                        nc.scalar.activation(fc[:, dt, :], ps, Copy)

                    # K^T (all s) and Q^T (first half = query rows)
                    for w_sb, dst in (
                        (wk_sb, kt_sb[:, :, ssl]),
                        (wq_sb, qt_sb[:, :, ssl] if sc * SC_W < SQ else None),
                    ):
                        if dst is None:
                            continue
                        for ot in range(4):
                            ps = psP.tile([128, SC_W], f32, tag="mm", bufs=3)
                            for dt in range(4):
                                nc.tensor.matmul(
                                    ps, w_sb[:, dt, ot * 128 : (ot + 1) * 128],
                                    fc[:, dt, :], start=(dt == 0), stop=(dt == 3),
                                )
                            nc.scalar.activation(dst[:, ot, :], ps, Copy)

                    # V rows for this s chunk (expr = contraction tiles 4..7)
                    for st in range(SC_W // 128):
                        sidx = sc * (SC_W // 128) + st
                        ps = psP.tile([128, D], f32, tag="mmv")
                        for dt in range(4):
                            nc.tensor.matmul(
                                ps, xt_c[:, 4 + dt, st * 128 : (st + 1) * 128],
                                wv_sb[:, dt, :], start=(dt == 0), stop=(dt == 3),
                            )
                        nc.scalar.activation(
                            v_sb[:, sidx, :, 0:HD],
                            ps.rearrange("p (h d) -> p h d", h=H),
                            Copy,
                        )

                    proj_att_step(sc)

            # ---------------- attention phase (remainder) ----------------
            with (
                tc.tile_pool(name="psB1", bufs=1, space="PSUM") as psB1,
                tc.tile_pool(name="psAV1", bufs=1, space="PSUM") as psAV1,
                tc.tile_pool(name="psO", bufs=1, space="PSUM") as psO,
                tc.tile_pool(name="psC", bufs=1, space="PSUM") as psC_,
            ):
                pools["tp"] = psC_
                pools["sc1"] = psB1
                pools["av1"] = psAV1
                pools["o"] = psO

                # finish head (0,0): last ktp step
                att_step(0, 0, 0, N_KTP - 1, av00)

                ctxs = {0: ctx0}
                for qc in range(N_QC):
                    if qc not in ctxs:
                        ctxs[qc] = make_ctx(qc)
                    ctx = ctxs[qc]
                    for hp in range(4):
                        done0 = qc == 0 and hp == 0
                        if done0:
                            av_ps0 = av00
                        else:
                            av_ps0 = psAV0.tile(
                                [HD + 1, QC_W], f32, tag="av0",
                                name=f"av0_{qc}{hp}",
                            )
                        av_ps1 = psAV1.tile(
                            [HD + 1, QC_W], f32, tag="av1", name=f"av1_{qc}{hp}"
                        )
                        for ktp in range(N_KTP):
                            for j in range(2):
                                if j == 0 and done0:
                                    continue
                                att_step(
                                    qc, hp, j, ktp,
                                    av_ps0 if j == 0 else av_ps1,
                                )
                            if qc == 0 and hp == 3 and ktp % 2 == 1:
                                load_mt_quarter(1, ktp // 2)
                        # epilogue must drain after this head pair's last
                        # (lagged) AV matmuls have been emitted
                        epi_due = sched["iter"] + AV_LAG + 1
                        sched["epi"].extend(
                            (epi_due, p)
                            for p in mk_epilogue(qc, hp, av_ps0, av_ps1, ctx)
                        )
                flush_all()

    nc.compile()
    return nc


def _get_prog():
    global _PROG
    if _PROG is None:
        _PROG = _build_program()
    return _PROG


def kernel(**inputs) -> np.ndarray:
    from concourse.bass_utils import run_bass_kernel_spmd
    from concourse import mybir

    bf16np = mybir.dt.np(mybir.dt.bfloat16)

    f = lambda k: np.asarray(inputs[k], dtype=np.float32)
    gene, expr, M = f("gene_emb"), f("expr_emb"), f("M")
    W_fused = f("W_fused")
    W_Q, W_K, W_V, W_O = f("W_Q"), f("W_K"), f("W_V"), f("W_O")
    b_O = f("b_O")

    scale = np.float32(HD ** -0.5)
    weights = dict(
        WF=np.ascontiguousarray(W_fused),
        WQ=np.ascontiguousarray(W_Q * scale),
        WK=np.ascontiguousarray(W_K),
        WV=np.ascontiguousarray(W_V),
        WO=np.ascontiguousarray(W_O).astype(bf16np),
    )

    nc = _get_prog()

    in_maps = []
    for c in range(8):
        b, qh = c // 2, c % 2
        xt = np.concatenate([gene[b], expr[b]], axis=1).T  # [1024, 2048]
        mt = M[b, qh * SQ : (qh + 1) * SQ, :].T            # [2048, 1024]
        if qh == 1:
            # permute sequence so this core's queries are s[0:1024]
            xt = np.concatenate([xt[:, SQ:], xt[:, :SQ]], axis=1)
            mt = np.concatenate([mt[SQ:], mt[:SQ]], axis=0)
        in_maps.append(
            dict(
                XT=np.ascontiguousarray(xt),
                MT=np.ascontiguousarray(mt).astype(bf16np),
                **weights,
            )
        )

    res = run_bass_kernel_spmd(nc, in_maps, core_ids=list(range(8)))

    out = np.empty((B, S, D), dtype=np.float32)
    for c in range(8):
        b, qh = c // 2, c % 2
        out[b, qh * SQ : (qh + 1) * SQ, :] = res.results[c]["OUT"] + b_O[None, :]
    return out


# revision 68
# speedup vs baseline: 1.0069x; 1.0018x over previous
"""Trainium2 Bass kernel for NewExpressionAttentionLayer (sparse gated attention).

Math (per batch b):
  fused = concat(gene, expr) @ W_fused
  Q = split(fused @ (W_Q*scale)); K = split(fused @ W_K); V = split(expr @ W_V)
  t = (Q K^T) * M          (scale folded into W_Q; M = gate)
  p = exp(t)               (softmax without max-subtraction; |t| <~ 6)
  pm = p * M
  A_bar = pm / sum_k(pm)   (softmax Z cancels; EPS term is O(1e-8) relative -> dropped)
  out = (A_bar @ V) @ W_O + b_O

Sharding: 8 cores = 4 batches x 2 query-halves. Each core computes its batch's
projections over all S (needed for K/V) and attention for its 1024 query rows.
For the second query half, the host permutes the sequence axis (swap halves) so
the device program always attends queries s[0:1024] — sums over k are
permutation-invariant.

Device layout is feature-major ("transposed"): activations [feat, seq] so the
PE (which contracts along partitions) needs no on-device transposes. The host
supplies X^T and M^T slices. Scores are computed transposed: scoresT[k, q] =
K^T_h.T @ Q^T_h.

Engine constraints that shape the schedule: only DVE and Act can read PSUM
(GPSIMD/Pool is SBUF-only, DMA cannot touch PSUM, matmul output is fp32
PSUM). So u = scores*M runs on DVE (f32, 1024-wide kt-pair ops); exp on
Act; pm = e*M (all-bf16 SBUF, 2x mode on DVE) alternates per head parity
between DVE and Pool; the per-query normalization of the output projection
alternates between a DVE scalar_tensor_tensor and an Act
per-partition-scale + Pool add. Per-head Z sums ride as a ones-column in
the V matmul, land on one PSUM row, and transpose to q-partition columns
via contraction-1 PE matmuls.

Two heads' chains are interleaved, AV matmuls are emitted several score
steps late (uniform lag, FIFO) so slow Pool multiplies never block the PE
wait queue, and a retired head pair's epilogue (copies, Z transpose,
output projection + normalize) is split into pieces drained one per step.
Head (0, qc0)'s chain is additionally interleaved into the projection
phase, where DVE/Act/Pool are otherwise idle.
"""

import sys

sys.path.insert(0, "/opt/trn_rl_repo")

import numpy as np

B, S, D = 4, 2048, 512
H, HD = 8, 64
SQ = S // 2          # query rows per core
KT_TILES = S // 128  # 16 k partition tiles
N_KTP = KT_TILES // 2  # 8 kt pairs
QC_W = 512           # q chunk width
N_QC = SQ // QC_W    # 2
SC_W = 256           # s chunk width for projections
N_SC = S // SC_W     # 8
NQT = QC_W // 128    # 4 query tiles per chunk

AV_LAG = 11           # score steps between pm emission and its AV matmuls

_PROG = None


def _build_program():
    from concourse import bacc, mybir
    import concourse.tile as tile

    f32 = mybir.dt.float32
    f32r = mybir.dt.float32r
    bf16 = mybir.dt.bfloat16
    Exp = mybir.ActivationFunctionType.Exp
    Copy = mybir.ActivationFunctionType.Copy
    MUL = mybir.AluOpType.mult
    ADD = mybir.AluOpType.add

    nc = bacc.Bacc("TRN2", target_bir_lowering=False, debug=False, num_devices=8)

    XT = nc.dram_tensor("XT", [2 * D, S], f32r, kind="ExternalInput").ap()
    MT = nc.dram_tensor("MT", [S, SQ], bf16, kind="ExternalInput").ap()
    WF = nc.dram_tensor("WF", [2 * D, D], f32r, kind="ExternalInput").ap()
    WQ = nc.dram_tensor("WQ", [D, D], f32r, kind="ExternalInput").ap()
    WK = nc.dram_tensor("WK", [D, D], f32r, kind="ExternalInput").ap()
    WV = nc.dram_tensor("WV", [D, D], f32r, kind="ExternalInput").ap()
    WO = nc.dram_tensor("WO", [D, D], bf16, kind="ExternalInput").ap()
    OUT = nc.dram_tensor("OUT", [SQ, D], f32, kind="ExternalOutput").ap()

    with tile.TileContext(nc) as tc:
        with (
            tc.tile_pool(name="misc", bufs=1) as misc,
            tc.tile_pool(name="kqv", bufs=1) as kqv,
            tc.tile_pool(name="mtp", bufs=1) as mtp,
            tc.tile_pool(name="att1", bufs=1) as att1,
            tc.tile_pool(name="att2", bufs=1) as att2,
            tc.tile_pool(name="wku", bufs=3) as wku,
            tc.tile_pool(name="wke", bufs=2) as wke,
            tc.tile_pool(name="wkp", bufs=2) as wkp,
            tc.tile_pool(name="psB0", bufs=1, space="PSUM") as psB0,
            tc.tile_pool(name="psAV0", bufs=1, space="PSUM") as psAV0,
        ):
            onecol = misc.tile([128, 1], bf16)
            nc.vector.memset(onecol, 1.0)
            one32 = misc.tile([1, 1], f32)
            nc.vector.memset(one32, 1.0)
            wo_sb = misc.tile([128, 4, D], bf16)

            kt_sb = kqv.tile([128, 4, S], bf16)     # K^T  [d, s]
            qt_sb = kqv.tile([128, 4, SQ], bf16)    # Q^T  [d, q]
            v_sb = kqv.tile([128, KT_TILES, H, HD + 1], bf16)  # V + ones col
            nc.vector.tensor_copy(
                v_sb[:, :, :, HD : HD + 1],
                onecol[:, None, :].broadcast_to([128, KT_TILES, H, 1]),
            )

            mt_r = MT.rearrange("(t p) q -> p t q", p=128)
            # the gate streams through a 4-deep ring of quarter tiles
            # (4 kt tiles each) so qc1's quarters prefetch while qc0 retires
            mt_quarters = {}
            qsls = [slice(0, QC_W), slice(QC_W, 2 * QC_W)]

            def load_mt_quarter(qc, q4):
                t = mtp.tile(
                    [128, 4, QC_W], bf16, tag="mt", name=f"mt{qc}_{q4}", bufs=4
                )
                nc.sync.dma_start(
                    out=t, in_=mt_r[:, q4 * 4 : (q4 + 1) * 4, qsls[qc]]
                )
                mt_quarters[(qc, q4)] = t

            def make_ctx(qc):
                outt = att1.tile(
                    [128, 4, QC_W], bf16, tag="outt", name=f"outt{qc}"
                )
                zr = att1.tile([1, H, QC_W], f32, tag="zr", name=f"zr{qc}")
                ps_t = [None]  # allocated lazily (PSUM pool opens post-proj)
                invt = att2.tile(
                    [128, NQT, H], f32, tag="invt", name=f"invt{qc}"
                )
                fin = [
                    att2.tile([128, D], f32, tag=f"fin{qtl}", name=f"fin{qc}_{qtl}")
                    for qtl in range(NQT)
                ]
                return (outt, zr, ps_t, invt, fin)

            # score-ring pools per head parity; head-1 ring + outproj PSUM
            # open only after the projection pools close (PSUM budget: during
            # proj psP(5) + psB0(2) + psAV0(1) = 8; after proj psB0(2) +
            # psAV0(1) + psB1(2) + psAV1(1) + psO(2) = 8)
            pools = {"sc0": psB0, "av0": psAV0, "sc1": None, "av1": None,
                     "o": None, "tp": None}

            # work FIFOs; the global step counter advances per score step
            sched = {"iter": 0, "av": [], "epi": [], "stt": 0}

            def drain():
                it = sched["iter"]
                while sched["av"] and sched["av"][0][0] <= it:
                    sched["av"].pop(0)[1]()
                if sched["epi"] and sched["epi"][0][0] <= it:
                    sched["epi"].pop(0)[1]()

            def flush_all():
                for _, f in sched["av"]:
                    f()
                sched["av"] = []
                for _, f in sched["epi"]:
                    f()
                sched["epi"] = []

            def mk_av(av_ps, ph, kt, ppm, i):
                def f():
                    nc.tensor.matmul(
                        av_ps, v_sb[:, kt, ph, :], ppm[:, i, :],
                        start=(kt == 0), stop=(kt == KT_TILES - 1),
                    )
                return f

            def att_step(qc, hp, j, ktp, av_ps):
                qsl = qsls[qc]
                p0, p1 = (0, 64) if j == 0 else (64, 128)
                h = 2 * hp + j
                off = (2 * ktp) % 4
                mt2 = mt_quarters[(qc, ktp // 2)][:, off : off + 2, :]
                scj = pools[f"sc{j}"].tile(
                    [128, 2, QC_W], f32, tag=f"sc{j}", name=f"sc{j}"
                )
                for i in range(2):
                    kt = 2 * ktp + i
                    nc.tensor.matmul(
                        scj[:, i, :],
                        kt_sb[p0:p1, hp, kt * 128 : (kt + 1) * 128],
                        qt_sb[p0:p1, hp, qsl],
                        start=True, stop=True,
                    )
                sched["iter"] += 1
                drain()
                u = wku.tile([128, 2, QC_W], f32, tag=f"u{j}", name=f"u{j}")
                nc.vector.tensor_mul(u, scj, mt2)
                e = wke.tile([128, 2, QC_W], bf16, tag=f"e{j}", name=f"e{j}")
                nc.scalar.activation(e, u, Exp)
                pm = wkp.tile([128, 2, QC_W], bf16, tag=f"pm{j}", name=f"pm{j}")
                # pm per head parity: even heads on DVE (2x bf16), odd on Pool
                if j == 0:
                    nc.vector.tensor_mul(pm, e, mt2)
                else:
                    nc.gpsimd.tensor_mul(pm, e, mt2)
                due = sched["iter"] + AV_LAG
                for i in range(2):
                    sched["av"].append((due, mk_av(av_ps, h, 2 * ktp + i, pm, i)))

            def mk_epilogue(qc, hp, ps_av0, ps_av1, ctx):
                outt, zr, ps_t, invt, fin = ctx

                def piece0():
                    if ps_t[0] is None:
                        ps_t[0] = pools["tp"].tile(
                            [128, NQT, H], f32, tag="tp", name=f"ps_t{qc}"
                        )
                    pst = ps_t[0]
                    for j, av_ps in ((0, ps_av0), (1, ps_av1)):
                        h = 2 * hp + j
                        nc.scalar.activation(
                            outt[j * 64 : j * 64 + 64, hp, :], av_ps[0:HD, :], Copy
                        )
                        nc.scalar.activation(
                            zr[0:1, h, :], av_ps[HD : HD + 1, :], Copy
                        )
                    # row->column transpose via contraction-1 matmuls
                    for j in range(2):
                        h = 2 * hp + j
                        for qtl in range(NQT):
                            nc.tensor.matmul(
                                pst[:, qtl, h : h + 1],
                                zr[0:1, h, qtl * 128 : (qtl + 1) * 128],
                                one32, start=True, stop=True,
                            )
                    nc.vector.reciprocal(
                        invt[:, :, 2 * hp : 2 * hp + 2],
                        pst[:, :, 2 * hp : 2 * hp + 2],
                    )

                pieces = [piece0]
                for j in range(2):
                    for qtl in range(NQT):
                        def piece(j=j, qtl=qtl):
                            h = 2 * hp + j
                            hoff = j * 64
                            qt_g = qc * NQT + qtl
                            ps_o = pools["o"].tile(
                                [128, D], f32, tag="o", name="ps_o"
                            )
                            nc.tensor.matmul(
                                ps_o,
                                outt[hoff : hoff + 64, hp,
                                     qtl * 128 : (qtl + 1) * 128],
                                wo_sb[hoff : hoff + 64, hp, :],
                                start=True, stop=True,
                            )
                            inv = invt[:, qtl, h : h + 1]
                            via_act = sched["stt"] % 2 == 1
                            sched["stt"] += 1
                            if h == 0:
                                if via_act:
                                    nc.scalar.mul(fin[qtl], ps_o, inv)
                                else:
                                    nc.vector.tensor_scalar_mul(
                                        fin[qtl], ps_o, inv
                                    )
                            elif via_act:
                                tmp = att2.tile(
                                    [128, D], bf16, tag="tmp", name="tmp"
                                )
                                nc.scalar.mul(tmp, ps_o, inv)
                                nc.gpsimd.tensor_add(fin[qtl], fin[qtl], tmp)
                            else:
                                nc.vector.scalar_tensor_tensor(
                                    out=fin[qtl], in0=ps_o, scalar=inv,
                                    in1=fin[qtl], op0=MUL, op1=ADD,
                                )
                            if h == H - 1:
                                nc.sync.dma_start(
                                    out=OUT[qt_g * 128 : (qt_g + 1) * 128, :],
                                    in_=fin[qtl],
                                )
                        pieces.append(piece)
                return pieces

            # ---------------- projection phase ----------------
            # head (hp0, j0) attention for qc0 interleaves with projection:
            # after chunk c, kt tiles 2c,2c+1 (and matching V rows) exist, so
            # its ktp step c-1 is safe (qt cols for qc0 exist after chunk 1)
            ctx0 = make_ctx(0)
            av00 = psAV0.tile([HD + 1, QC_W], f32, tag="av0", name="av0_00")

            def proj_att_step(c):
                ktp = c - 1
                if ktp < 0 or ktp >= N_KTP - 1:
                    return
                att_step(0, 0, 0, ktp, av00)

            with (
                tc.tile_pool(name="projw", bufs=1) as projw,
                tc.tile_pool(name="xtp", bufs=2) as xtp,
                tc.tile_pool(name="fcp", bufs=2) as fcp,
                tc.tile_pool(name="psP", bufs=2, space="PSUM") as psP,
            ):
                xt_r = XT.rearrange("(t p) s -> p t s", p=128)
                wf_sb = projw.tile([128, 8, D], f32r)
                wf_r = WF.rearrange("(t p) n -> p t n", p=128)
                # first fused matmul needs wf[t] + xt0[t] in stream order
                xt_c0 = xtp.tile([128, 8, SC_W], f32r, tag="xt")
                for t in range(8):
                    nc.sync.dma_start(out=wf_sb[:, t, :], in_=wf_r[:, t, :])
                    nc.sync.dma_start(out=xt_c0[:, t, :], in_=xt_r[:, t, 0:SC_W])
                wk_sb = projw.tile([128, 4, D], f32r)
                nc.sync.dma_start(out=wk_sb, in_=WK.rearrange("(t p) n -> p t n", p=128))
                wq_sb = projw.tile([128, 4, D], f32r)
                nc.sync.dma_start(out=wq_sb, in_=WQ.rearrange("(t p) n -> p t n", p=128))
                wv_sb = projw.tile([128, 4, D], f32r)
                nc.sync.dma_start(out=wv_sb, in_=WV.rearrange("(t p) n -> p t n", p=128))
                xt_c1 = xtp.tile([128, 8, SC_W], f32r, tag="xt")
                nc.sync.dma_start(out=xt_c1, in_=xt_r[:, :, SC_W : 2 * SC_W])
                nc.sync.dma_start(out=wo_sb, in_=WO.rearrange("(t p) n -> p t n", p=128))

                for sc in range(N_SC):
                    ssl = slice(sc * SC_W, (sc + 1) * SC_W)
                    if sc == 0:
                        xt_c = xt_c0
                    elif sc == 1:
                        xt_c = xt_c1
                    else:
                        xt_c = xtp.tile([128, 8, SC_W], f32r, tag="xt")
                        nc.sync.dma_start(out=xt_c, in_=xt_r[:, :, ssl])
                    # stream a quarter of the qc0 gate right behind each
                    # even chunk's xt load (quarter q4 feeds ktp 2*q4+..)
                    if sc % 2 == 0 and sc < 8:
                        load_mt_quarter(0, sc // 2)

                    proj_att_step(sc)

                    fc = fcp.tile([128, 4, SC_W], f32r, tag="fc")
                    for dt in range(4):
                        ps = psP.tile([128, SC_W], f32, tag="mm", bufs=3)
                        for t in range(8):
                            nc.tensor.matmul(
                                ps, wf_sb[:, t, dt * 128 : (dt + 1) * 128],
                                xt_c[:, t, :], start=(t == 0), stop=(t == 7),
                            )
                        nc.scalar.activation(fc[:, dt, :], ps, Copy)

                    # K^T (all s) and Q^T (first half = query rows)
                    for w_sb, dst in (
                        (wk_sb, kt_sb[:, :, ssl]),
                        (wq_sb, qt_sb[:, :, ssl] if sc * SC_W < SQ else None),
                    ):
                        if dst is None:
                            continue
                        for ot in range(4):
                            ps = psP.tile([128, SC_W], f32, tag="mm", bufs=3)
                            for dt in range(4):
                                nc.tensor.matmul(
                                    ps, w_sb[:, dt, ot * 128 : (ot + 1) * 128],
                                    fc[:, dt, :], start=(dt == 0), stop=(dt == 3),
                                )
                            nc.scalar.activation(dst[:, ot, :], ps, Copy)

                    # V rows for this s chunk (expr = contraction tiles 4..7)
                    for st in range(SC_W // 128):
                        sidx = sc * (SC_W // 128) + st
                        ps = psP.tile([128, D], f32, tag="mmv")
                        for dt in range(4):
                            nc.tensor.matmul(
                                ps, xt_c[:, 4 + dt, st * 128 : (st + 1) * 128],
                                wv_sb[:, dt, :], start=(dt == 0), stop=(dt == 3),
                            )
                        nc.scalar.activation(
                            v_sb[:, sidx, :, 0:HD],
                            ps.rearrange("p (h d) -> p h d", h=H),
                            Copy,
                        )

            # ---------------- attention phase (remainder) ----------------
            with (
                tc.tile_pool(name="psB1", bufs=1, space="PSUM") as psB1,
                tc.tile_pool(name="psAV1", bufs=1, space="PSUM") as psAV1,
                tc.tile_pool(name="psO", bufs=1, space="PSUM") as psO,
                tc.tile_pool(name="psC", bufs=1, space="PSUM") as psC_,
            ):
                pools["tp"] = psC_
                pools["sc1"] = psB1
                pools["av1"] = psAV1
                pools["o"] = psO

                # finish head (0,0): last ktp step
                att_step(0, 0, 0, N_KTP - 1, av00)

                ctxs = {0: ctx0}
                for qc in range(N_QC):
                    if qc not in ctxs:
                        ctxs[qc] = make_ctx(qc)
                    ctx = ctxs[qc]
                    for hp in range(4):
                        done0 = qc == 0 and hp == 0
                        if done0:
                            av_ps0 = av00
                        else:
                            av_ps0 = psAV0.tile(
                                [HD + 1, QC_W], f32, tag="av0",
                                name=f"av0_{qc}{hp}",
                            )
                        av_ps1 = psAV1.tile(
                            [HD + 1, QC_W], f32, tag="av1", name=f"av1_{qc}{hp}"
                        )
                        for ktp in range(N_KTP):
                            for j in range(2):
                                if j == 0 and done0:
                                    continue
                                att_step(
                                    qc, hp, j, ktp,
                                    av_ps0 if j == 0 else av_ps1,
                                )
                            if qc == 0 and hp == 3 and ktp % 2 == 1:
                                load_mt_quarter(1, ktp // 2)
                        # epilogue must drain after this head pair's last
                        # (lagged) AV matmuls have been emitted
                        epi_due = sched["iter"] + AV_LAG + 1
                        sched["epi"].extend(
                            (epi_due, p)
                            for p in mk_epilogue(qc, hp, av_ps0, av_ps1, ctx)
                        )
                flush_all()

    nc.compile()
    return nc


def _get_prog():
    global _PROG
    if _PROG is None:
        _PROG = _build_program()
    return _PROG


def kernel(**inputs) -> np.ndarray:
    from concourse.bass_utils import run_bass_kernel_spmd
    from concourse import mybir

    bf16np = mybir.dt.np(mybir.dt.bfloat16)

    f = lambda k: np.asarray(inputs[k], dtype=np.float32)
    gene, expr, M = f("gene_emb"), f("expr_emb"), f("M")
    W_fused = f("W_fused")
    W_Q, W_K, W_V, W_O = f("W_Q"), f("W_K"), f("W_V"), f("W_O")
    b_O = f("b_O")

    scale = np.float32(HD ** -0.5)
    weights = dict(
        WF=np.ascontiguousarray(W_fused),
        WQ=np.ascontiguousarray(W_Q * scale),
        WK=np.ascontiguousarray(W_K),
        WV=np.ascontiguousarray(W_V),
        WO=np.ascontiguousarray(W_O).astype(bf16np),
    )

    nc = _get_prog()

    in_maps = []
    for c in range(8):
        b, qh = c // 2, c % 2
        xt = np.concatenate([gene[b], expr[b]], axis=1).T  # [1024, 2048]
        mt = M[b, qh * SQ : (qh + 1) * SQ, :].T            # [2048, 1024]
        if qh == 1:
            # permute sequence so this core's queries are s[0:1024]
            xt = np.concatenate([xt[:, SQ:], xt[:, :SQ]], axis=1)
            mt = np.concatenate([mt[SQ:], mt[:SQ]], axis=0)
        in_maps.append(
            dict(
                XT=np.ascontiguousarray(xt),
                MT=np.ascontiguousarray(mt).astype(bf16np),
                **weights,
            )
        )

    res = run_bass_kernel_spmd(nc, in_maps, core_ids=list(range(8)))

    out = np.empty((B, S, D), dtype=np.float32)
    for c in range(8):
        b, qh = c // 2, c % 2
        out[b, qh * SQ : (qh + 1) * SQ, :] = res.results[c]["OUT"] + b_O[None, :]
    return out


# revision 73
# speedup vs baseline: 1.0094x; 1.0025x over previous
"""Trainium2 Bass kernel for NewExpressionAttentionLayer (sparse gated attention).

Math (per batch b):
  fused = concat(gene, expr) @ W_fused
  Q = split(fused @ (W_Q*scale)); K = split(fused @ W_K); V = split(expr @ W_V)
  t = (Q K^T) * M          (scale folded into W_Q; M = gate)
  p = exp(t)               (softmax without max-subtraction; |t| <~ 6)
  pm = p * M
  A_bar = pm / sum_k(pm)   (softmax Z cancels; EPS term is O(1e-8) relative -> dropped)
  out = (A_bar @ V) @ W_O + b_O

Sharding: 8 cores = 4 batches x 2 query-halves. Each core computes its batch's
projections over all S (needed for K/V) and attention for its 1024 query rows.
For the second query half, the host permutes the sequence axis (swap halves) so
the device program always attends queries s[0:1024] — sums over k are
permutation-invariant.

Device layout is feature-major ("transposed"): activations [feat, seq] so the
PE (which contracts along partitions) needs no on-device transposes. The host
supplies X^T and M^T slices. Scores are computed transposed: scoresT[k, q] =
K^T_h.T @ Q^T_h.

Engine constraints that shape the schedule: only DVE and Act can read PSUM
(GPSIMD/Pool is SBUF-only, DMA cannot touch PSUM, matmul output is fp32
PSUM). So u = scores*M runs on DVE (f32, 1024-wide kt-pair ops); exp on
Act; pm = e*M (all-bf16 SBUF, 2x mode on DVE) alternates per head parity
between DVE and Pool; the per-query normalization of the output projection
alternates between a DVE scalar_tensor_tensor and an Act
per-partition-scale + Pool add. Per-head Z sums ride as a ones-column in
the V matmul, land on one PSUM row, and transpose to q-partition columns
via contraction-1 PE matmuls.

Two heads' chains are interleaved, AV matmuls are emitted several score
steps late (uniform lag, FIFO) so slow Pool multiplies never block the PE
wait queue, and a retired head pair's epilogue (copies, Z transpose,
output projection + normalize) is split into pieces drained one per step.
Head (0, qc0)'s chain is additionally interleaved into the projection
phase, where DVE/Act/Pool are otherwise idle.
"""

import sys

sys.path.insert(0, "/opt/trn_rl_repo")

import numpy as np

B, S, D = 4, 2048, 512
H, HD = 8, 64
SQ = S // 2          # query rows per core
KT_TILES = S // 128  # 16 k partition tiles
N_KTP = KT_TILES // 2  # 8 kt pairs
QC_W = 512           # q chunk width
N_QC = SQ // QC_W    # 2
SC_W = 256           # s chunk width for projections
N_SC = S // SC_W     # 8
NQT = QC_W // 128    # 4 query tiles per chunk

AV_LAG = 11           # score steps between pm emission and its AV matmuls

_PROG = None


def _build_program():
    from concourse import bacc, mybir
    import concourse.tile as tile

    f32 = mybir.dt.float32
    f32r = mybir.dt.float32r
    bf16 = mybir.dt.bfloat16
    Exp = mybir.ActivationFunctionType.Exp
    Copy = mybir.ActivationFunctionType.Copy
    MUL = mybir.AluOpType.mult
    ADD = mybir.AluOpType.add

    nc = bacc.Bacc("TRN2", target_bir_lowering=False, debug=False, num_devices=8)

    XT = nc.dram_tensor("XT", [2 * D, S], f32r, kind="ExternalInput").ap()
    MT = nc.dram_tensor("MT", [S, SQ], bf16, kind="ExternalInput").ap()
    WF = nc.dram_tensor("WF", [2 * D, D], f32r, kind="ExternalInput").ap()
    WQ = nc.dram_tensor("WQ", [D, D], f32r, kind="ExternalInput").ap()
    WK = nc.dram_tensor("WK", [D, D], f32r, kind="ExternalInput").ap()
    WV = nc.dram_tensor("WV", [D, D], f32r, kind="ExternalInput").ap()
    WO = nc.dram_tensor("WO", [D, D], bf16, kind="ExternalInput").ap()
    OUT = nc.dram_tensor("OUT", [SQ, D], f32, kind="ExternalOutput").ap()

    with tile.TileContext(nc) as tc:
        with (
            tc.tile_pool(name="misc", bufs=1) as misc,
            tc.tile_pool(name="kqv", bufs=1) as kqv,
            tc.tile_pool(name="mtp", bufs=1) as mtp,
            tc.tile_pool(name="att1", bufs=1) as att1,
            tc.tile_pool(name="att2", bufs=1) as att2,
            tc.tile_pool(name="wku", bufs=3) as wku,
            tc.tile_pool(name="wke", bufs=2) as wke,
            tc.tile_pool(name="wkp", bufs=2) as wkp,
            tc.tile_pool(name="psB0", bufs=1, space="PSUM") as psB0,
            tc.tile_pool(name="psAV0", bufs=1, space="PSUM") as psAV0,
        ):
            onecol = misc.tile([128, 1], bf16)
            nc.vector.memset(onecol, 1.0)
            one32 = misc.tile([1, 1], f32)
            nc.vector.memset(one32, 1.0)
            wo_sb = misc.tile([128, 4, D], bf16)

            kt_sb = kqv.tile([128, 4, S], bf16)     # K^T  [d, s]
            qt_sb = kqv.tile([128, 4, SQ], bf16)    # Q^T  [d, q]
            v_sb = kqv.tile([128, KT_TILES, H, HD + 1], bf16)  # V + ones col
            nc.vector.tensor_copy(
                v_sb[:, :, :, HD : HD + 1],
                onecol[:, None, :].broadcast_to([128, KT_TILES, H, 1]),
            )

            mt_r = MT.rearrange("(t p) q -> p t q", p=128)
            # the gate streams through a 4-deep ring of quarter tiles
            # (4 kt tiles each) so qc1's quarters prefetch while qc0 retires
            mt_quarters = {}
            qsls = [slice(0, QC_W), slice(QC_W, 2 * QC_W)]

            def load_mt_quarter(qc, q4):
                t = mtp.tile(
                    [128, 4, QC_W], bf16, tag="mt", name=f"mt{qc}_{q4}", bufs=4
                )
                nc.sync.dma_start(
                    out=t, in_=mt_r[:, q4 * 4 : (q4 + 1) * 4, qsls[qc]]
                )
                mt_quarters[(qc, q4)] = t

            def make_ctx(qc):
                outt = att1.tile(
                    [128, 4, QC_W], bf16, tag="outt", name=f"outt{qc}"
                )
                zr = att1.tile([1, H, QC_W], f32, tag="zr", name=f"zr{qc}")
                ps_t = [None]  # allocated lazily (PSUM pool opens post-proj)
                invt = att2.tile(
                    [128, NQT, H], f32, tag="invt", name=f"invt{qc}"
                )
                fin = [
                    att2.tile([128, D], f32, tag=f"fin{qtl}", name=f"fin{qc}_{qtl}")
                    for qtl in range(NQT)
                ]
                return (outt, zr, ps_t, invt, fin)

            # score-ring pools per head parity; head-1 ring + outproj PSUM
            # open only after the projection pools close (PSUM budget: during
            # proj psP(5) + psB0(2) + psAV0(1) = 8; after proj psB0(2) +
            # psAV0(1) + psB1(2) + psAV1(1) + psO(2) = 8)
            pools = {"sc0": psB0, "av0": psAV0, "sc1": None, "av1": None,
                     "o": None, "tp": None}

            # work FIFOs; the global step counter advances per score step
            sched = {"iter": 0, "av": [], "epi": [], "stt": 0}

            def drain():
                it = sched["iter"]
                while sched["av"] and sched["av"][0][0] <= it:
                    sched["av"].pop(0)[1]()
                if sched["epi"] and sched["epi"][0][0] <= it:
                    sched["epi"].pop(0)[1]()

            def flush_all():
                for _, f in sched["av"]:
                    f()
                sched["av"] = []
                for _, f in sched["epi"]:
                    f()
                sched["epi"] = []

            def mk_av(av_ps, ph, kt, ppm, i):
                def f():
                    nc.tensor.matmul(
                        av_ps, v_sb[:, kt, ph, :], ppm[:, i, :],
                        start=(kt == 0), stop=(kt == KT_TILES - 1),
                    )
                return f

            def att_step(qc, hp, j, ktp, av_ps):
                qsl = qsls[qc]
                p0, p1 = (0, 64) if j == 0 else (64, 128)
                h = 2 * hp + j
                off = (2 * ktp) % 4
                mt2 = mt_quarters[(qc, ktp // 2)][:, off : off + 2, :]
                scj = pools[f"sc{j}"].tile(
                    [128, 2, QC_W], f32, tag=f"sc{j}", name=f"sc{j}"
                )
                for i in range(2):
                    kt = 2 * ktp + i
                    nc.tensor.matmul(
                        scj[:, i, :],
                        kt_sb[p0:p1, hp, kt * 128 : (kt + 1) * 128],
                        qt_sb[p0:p1, hp, qsl],
                        start=True, stop=True,
                    )
                sched["iter"] += 1
                drain()
                u = wku.tile([128, 2, QC_W], f32, tag=f"u{j}", name=f"u{j}")
                nc.vector.tensor_mul(u, scj, mt2)
                e = wke.tile([128, 2, QC_W], bf16, tag=f"e{j}", name=f"e{j}")
                nc.scalar.activation(e, u, Exp)
                pm = wkp.tile([128, 2, QC_W], bf16, tag=f"pm{j}", name=f"pm{j}")
                # pm per head parity: even heads on DVE (2x bf16), odd on Pool
                if j == 0:
                    nc.vector.tensor_mul(pm, e, mt2)
                else:
                    nc.gpsimd.tensor_mul(pm, e, mt2)
                due = sched["iter"] + AV_LAG
                for i in range(2):
                    sched["av"].append((due, mk_av(av_ps, h, 2 * ktp + i, pm, i)))

            def mk_epilogue(qc, hp, ps_av0, ps_av1, ctx):
                outt, zr, ps_t, invt, fin = ctx

                def piece0():
                    if ps_t[0] is None:
                        ps_t[0] = pools["tp"].tile(
                            [128, NQT, H], f32, tag="tp", name=f"ps_t{qc}"
                        )
                    pst = ps_t[0]
                    for j, av_ps in ((0, ps_av0), (1, ps_av1)):
                        h = 2 * hp + j
                        nc.scalar.activation(
                            outt[j * 64 : j * 64 + 64, hp, :], av_ps[0:HD, :], Copy
                        )
                        nc.scalar.activation(
                            zr[0:1, h, :], av_ps[HD : HD + 1, :], Copy
                        )
                    # row->column transpose via contraction-1 matmuls
                    for j in range(2):
                        h = 2 * hp + j
                        for qtl in range(NQT):
                            nc.tensor.matmul(
                                pst[:, qtl, h : h + 1],
                                zr[0:1, h, qtl * 128 : (qtl + 1) * 128],
                                one32, start=True, stop=True,
                            )
                    nc.vector.reciprocal(
                        invt[:, :, 2 * hp : 2 * hp + 2],
                        pst[:, :, 2 * hp : 2 * hp + 2],
                    )

                pieces = [piece0]
                for j in range(2):
                    for qtl in range(NQT):
                        def piece(j=j, qtl=qtl):
                            h = 2 * hp + j
                            hoff = j * 64
                            qt_g = qc * NQT + qtl
                            ps_o = pools["o"].tile(
                                [128, D], f32, tag="o", name="ps_o"
                            )
                            nc.tensor.matmul(
                                ps_o,
                                outt[hoff : hoff + 64, hp,
                                     qtl * 128 : (qtl + 1) * 128],
                                wo_sb[hoff : hoff + 64, hp, :],
                                start=True, stop=True,
                            )
                            inv = invt[:, qtl, h : h + 1]
                            via_act = sched["stt"] % 2 == 1
                            sched["stt"] += 1
                            if h == 0:
                                if via_act:
                                    nc.scalar.mul(fin[qtl], ps_o, inv)
                                else:
                                    nc.vector.tensor_scalar_mul(
                                        fin[qtl], ps_o, inv
                                    )
                            elif via_act:
                                tmp = att2.tile(
                                    [128, D], bf16, tag="tmp", name="tmp"
                                )
                                nc.scalar.mul(tmp, ps_o, inv)
                                nc.gpsimd.tensor_add(fin[qtl], fin[qtl], tmp)
                            else:
                                nc.vector.scalar_tensor_tensor(
                                    out=fin[qtl], in0=ps_o, scalar=inv,
                                    in1=fin[qtl], op0=MUL, op1=ADD,
                                )
                            if h == H - 1:
                                nc.sync.dma_start(
                                    out=OUT[qt_g * 128 : (qt_g + 1) * 128, :],
                                    in_=fin[qtl],
                                )
                        pieces.append(piece)
                return pieces

            # ---------------- projection phase ----------------
            # head (hp0, j0) attention for qc0 interleaves with projection:
            # after chunk c, kt tiles 2c,2c+1 (and matching V rows) exist, so
            # its ktp step c-1 is safe (qt cols for qc0 exist after chunk 1)
            ctx0 = make_ctx(0)
            av00 = psAV0.tile([HD + 1, QC_W], f32, tag="av0", name="av0_00")

            def proj_att_step(c):
                ktp = c - 1
                if ktp < 0 or ktp >= N_KTP - 1:
                    return
                att_step(0, 0, 0, ktp, av00)

            with (
                tc.tile_pool(name="projw", bufs=1) as projw,
                tc.tile_pool(name="xtp", bufs=2) as xtp,
                tc.tile_pool(name="fcp", bufs=2) as fcp,
                tc.tile_pool(name="psP", bufs=2, space="PSUM") as psP,
            ):
                xt_r = XT.rearrange("(t p) s -> p t s", p=128)
                wf_sb = projw.tile([128, 8, D], f32r)
                wf_r = WF.rearrange("(t p) n -> p t n", p=128)
                # first fused matmul needs wf[t] + xt0[t] in stream order
                xt_c0 = xtp.tile([128, 8, SC_W], f32r, tag="xt")
                for t in range(8):
                    nc.sync.dma_start(out=wf_sb[:, t, :], in_=wf_r[:, t, :])
                    nc.sync.dma_start(out=xt_c0[:, t, :], in_=xt_r[:, t, 0:SC_W])
                wk_sb = projw.tile([128, 4, D], f32r)
                nc.sync.dma_start(out=wk_sb, in_=WK.rearrange("(t p) n -> p t n", p=128))
                wq_sb = projw.tile([128, 4, D], f32r)
                nc.sync.dma_start(out=wq_sb, in_=WQ.rearrange("(t p) n -> p t n", p=128))
                wv_sb = projw.tile([128, 4, D], f32r)
                nc.sync.dma_start(out=wv_sb, in_=WV.rearrange("(t p) n -> p t n", p=128))
                xt_c1 = xtp.tile([128, 8, SC_W], f32r, tag="xt")
                nc.sync.dma_start(out=xt_c1, in_=xt_r[:, :, SC_W : 2 * SC_W])
                nc.sync.dma_start(out=wo_sb, in_=WO.rearrange("(t p) n -> p t n", p=128))

                for sc in range(N_SC):
                    ssl = slice(sc * SC_W, (sc + 1) * SC_W)
                    if sc == 0:
                        xt_c = xt_c0
                    elif sc == 1:
                        xt_c = xt_c1
                    else:
                        xt_c = xtp.tile([128, 8, SC_W], f32r, tag="xt")
                        nc.sync.dma_start(out=xt_c, in_=xt_r[:, :, ssl])
                    # stream a quarter of the qc0 gate right behind each
                    # even chunk's xt load (quarter q4 feeds ktp 2*q4+..)
                    if sc % 2 == 0 and sc < 8:
                        load_mt_quarter(0, sc // 2)

                    proj_att_step(sc)

                    fc = fcp.tile([128, 4, SC_W], f32r, tag="fc")
                    for dt in range(4):
                        ps = psP.tile([128, SC_W], f32, tag="mm", bufs=3)
                        for t in range(8):
                            nc.tensor.matmul(
                                ps, wf_sb[:, t, dt * 128 : (dt + 1) * 128],
                                xt_c[:, t, :], start=(t == 0), stop=(t == 7),
                            )
                        nc.scalar.activation(fc[:, dt, :], ps, Copy)

                    # K^T (all s) and Q^T (first half = query rows)
                    for w_sb, dst in (
                        (wk_sb, kt_sb[:, :, ssl]),
                        (wq_sb, qt_sb[:, :, ssl] if sc * SC_W < SQ else None),
                    ):
                        if dst is None:
                            continue
                        for ot in range(4):
                            ps = psP.tile([128, SC_W], f32, tag="mm", bufs=3)
                            for dt in range(4):
                                nc.tensor.matmul(
                                    ps, w_sb[:, dt, ot * 128 : (ot + 1) * 128],
                                    fc[:, dt, :], start=(dt == 0), stop=(dt == 3),
                                )
                            nc.scalar.activation(dst[:, ot, :], ps, Copy)

                    # V rows for this s chunk (expr = contraction tiles 4..7)
                    for st in range(SC_W // 128):
                        sidx = sc * (SC_W // 128) + st
                        ps = psP.tile([128, D], f32, tag="mmv")
                        for dt in range(4):
                            nc.tensor.matmul(
                                ps, xt_c[:, 4 + dt, st * 128 : (st + 1) * 128],
                                wv_sb[:, dt, :], start=(dt == 0), stop=(dt == 3),
                            )
                        nc.scalar.activation(
                            v_sb[:, sidx, :, 0:HD],
                            ps.rearrange("p (h d) -> p h d", h=H),
                            Copy,
                        )

            # ---------------- attention phase (remainder) ----------------
            with (
                tc.tile_pool(name="psB1", bufs=1, space="PSUM") as psB1,
                tc.tile_pool(name="psAV1", bufs=1, space="PSUM") as psAV1,
                tc.tile_pool(name="psO", bufs=1, space="PSUM") as psO,
                tc.tile_pool(name="psC", bufs=1, space="PSUM") as psC_,
            ):
                pools["tp"] = psC_
                pools["sc1"] = psB1
                pools["av1"] = psAV1
                pools["o"] = psO

                ctxs = {0: ctx0}
                for qc in range(N_QC):
                    if qc not in ctxs:
                        ctxs[qc] = make_ctx(qc)
                    ctx = ctxs[qc]
                    for hp in range(4):
                        done0 = qc == 0 and hp == 0
                        if done0:
                            av_ps0 = av00
                        else:
                            av_ps0 = psAV0.tile(
                                [HD + 1, QC_W], f32, tag="av0",
                                name=f"av0_{qc}{hp}",
                            )
                        av_ps1 = psAV1.tile(
                            [HD + 1, QC_W], f32, tag="av1", name=f"av1_{qc}{hp}"
                        )
                        for ktp in range(N_KTP):
                            for j in range(2):
                                if j == 0 and done0:
                                    continue
                                att_step(
                                    qc, hp, j, ktp,
                                    av_ps0 if j == 0 else av_ps1,
                                )
                            if done0 and ktp == 1:
                                # head-0's last step waits on chunk-7 copies;
                                # run it behind two ready j1 steps
                                att_step(0, 0, 0, N_KTP - 1, av00)
                            if qc == 0 and hp == 3 and ktp % 2 == 1:
                                load_mt_quarter(1, ktp // 2)
                        # epilogue must drain after this head pair's last
                        # (lagged) AV matmuls have been emitted
                        epi_due = sched["iter"] + AV_LAG + 1
                        sched["epi"].extend(
                            (epi_due, p)
                            for p in mk_epilogue(qc, hp, av_ps0, av_ps1, ctx)
                        )
                flush_all()

    nc.compile()
    return nc


def _get_prog():
    global _PROG
    if _PROG is None:
        _PROG = _build_program()
    return _PROG


def kernel(**inputs) -> np.ndarray:
    from concourse.bass_utils import run_bass_kernel_spmd
    from concourse import mybir

    bf16np = mybir.dt.np(mybir.dt.bfloat16)

    f = lambda k: np.asarray(inputs[k], dtype=np.float32)
    gene, expr, M = f("gene_emb"), f("expr_emb"), f("M")
    W_fused = f("W_fused")
    W_Q, W_K, W_V, W_O = f("W_Q"), f("W_K"), f("W_V"), f("W_O")
    b_O = f("b_O")

    scale = np.float32(HD ** -0.5)
    weights = dict(
        WF=np.ascontiguousarray(W_fused),
        WQ=np.ascontiguousarray(W_Q * scale),
        WK=np.ascontiguousarray(W_K),
        WV=np.ascontiguousarray(W_V),
        WO=np.ascontiguousarray(W_O).astype(bf16np),
    )

    nc = _get_prog()

    in_maps = []
    for c in range(8):
        b, qh = c // 2, c % 2
        xt = np.concatenate([gene[b], expr[b]], axis=1).T  # [1024, 2048]
        mt = M[b, qh * SQ : (qh + 1) * SQ, :].T            # [2048, 1024]
        if qh == 1:
            # permute sequence so this core's queries are s[0:1024]
            xt = np.concatenate([xt[:, SQ:], xt[:, :SQ]], axis=1)
            mt = np.concatenate([mt[SQ:], mt[:SQ]], axis=0)
        in_maps.append(
            dict(
                XT=np.ascontiguousarray(xt),
                MT=np.ascontiguousarray(mt).astype(bf16np),
                **weights,
            )
        )

    res = run_bass_kernel_spmd(nc, in_maps, core_ids=list(range(8)))

    out = np.empty((B, S, D), dtype=np.float32)
    for c in range(8):
        b, qh = c // 2, c % 2
        out[b, qh * SQ : (qh + 1) * SQ, :] = res.results[c]["OUT"] + b_O[None, :]
    return out
